# revision 24
# baseline (speedup 1.0000x reference)
"""Causal single-head attention (B=4, T=4096, C=1024, H=64) on 8 trn2 NeuronCores.

Sharding: core = (batch b = core//2, parity p = core%2). Each core owns the
interleaved context tiles {p, p+2, ...} of its batch (balanced under the causal
mask) and computes partial flash-attention (numerator + denominator) for ALL
queries of the batch; the host sums the two partials per batch and divides.

Host-side prep is layout only: the x shard is passed pre-transposed [C, T]
with columns ordered [own tiles | partner tiles] (pure permutation), weights
pre-fused into their SBUF layout, causal masks as data. All numerics run on
device. No collectives: each core projects the k-side for the full sequence
locally (trades HBM reads for the multi-10us cold-start of on-chip
collectives).

Device pipeline per core:
  load x^T slices (interleaved own/partner order) -> project [S^|G^] with a
  fused [wk|wq] fp32r stationary as slices land -> per 512-query block:
  row-packed bf16 score matmuls -> exp on ACT (scale folded in) -> data-driven
  causal masks on DVE -> bf16 PV matmuls (V padded to 128 cols for fast
  weight load) accumulating [V|1]^T @ P^T in dual PSUM chains -> write
  O^T_aug [H+1, T].

Query columns are processed in [own|partner] order per 512-block; the host
maps them back to absolute order per core in combine_outputs.
"""

import sys

for _p in ("/root/.axon_site/_ro/trn_rl_repo", "/root/.axon_site/_ro/pypackages"):
    if _p not in sys.path:
        sys.path.append(_p)

import ml_dtypes
import numpy as np

import concourse.bass as bass
import concourse.mybir as mybir
import concourse.tile as tile
from concourse import bacc
from concourse.bass_utils import run_bass_kernel_spmd
from concourse.masks import make_identity

B, T, C, H = 4, 4096, 1024, 64
N_CORES = 8
SCALE = C ** -0.5
F32 = mybir.dt.float32
F32R = mybir.dt.float32r
BF16 = mybir.dt.bfloat16


def build_kernel(t_full=T):
    """Build the SPMD Bass/Tile program for sequence length t_full."""
    t_own = t_full // 2           # context rows owned by this core
    n_own = t_own // 128          # own 128-row s-tiles
    n_blk = t_full // 512         # 512-wide query blocks
    n_cchunk = C // 128           # contraction chunks of 128
    n_tq = t_full // 512          # projection/load slices (stored order)
    n_vq = t_own // 512           # V projection slices (own region)

    nc = bacc.Bacc("TRN2", target_bir_lowering=False, debug=False,
                   num_devices=N_CORES)

    xt_d = nc.dram_tensor("xt_own", [C, t_full], BF16,
                      kind="ExternalInput").ap()
    aux_w = n_cchunk * 128 + n_cchunk * H + 1024
    aux_d = nc.dram_tensor("aux", [128, aux_w], BF16,
                           kind="ExternalInput").ap()
    bkq_d = nc.dram_tensor("bkq", [128], F32, kind="ExternalInput").ap()
    bv_d = nc.dram_tensor("bv", [64], F32, kind="ExternalInput").ap()
    out_d = nc.dram_tensor("out_part", [H + 1, t_full], F32,
                           kind="ExternalOutput").ap()

    with tile.TileContext(nc) as tc:
        with (
            tc.tile_pool(name="persist", bufs=1) as pp,
            tc.tile_pool(name="psB", bufs=2, space="PSUM") as psb,
            tc.tile_pool(name="psS", bufs=2, space="PSUM") as pss,
            tc.tile_pool(name="psOA", bufs=1, space="PSUM") as psoa,
            tc.tile_pool(name="psOB", bufs=1, space="PSUM") as psob,
            tc.tile_pool(name="ptp", bufs=4) as ptp,
            tc.tile_pool(name="outp", bufs=2) as outp,
        ):
            # ---- persistent SBUF tensors ----
            xt = pp.tile([128, n_cchunk * t_full], BF16)  # x^T, chunk j at cols t_full*j
            kqT = pp.tile([128, t_full], BF16)            # 0:64 = S^T, 64:128 = G^T
            sd_hi = pp.tile([128, t_full], BF16)          # S^T dup at partitions 64:128
            gt_lo = pp.tile([64, t_own], BF16)            # G^T (own) at partitions 0:64
            vT = pp.tile([64, t_own], F32)                # V^T own
            v_sb = pp.tile([128, n_own * 128], BF16)      # V_aug tiles [128,128] (padded)
            aux_sb = pp.tile([128, aux_w], BF16)
            wkq_sb = aux_sb[:, 0:n_cchunk * 128]
            wv_sb = aux_sb[:, n_cchunk * 128:n_cchunk * 128 + n_cchunk * H]
            mask0 = aux_sb[:, aux_w - 1024:aux_w - 512]
            mask1 = aux_sb[:, aux_w - 512:aux_w]
            bias_kq = pp.tile([128, 1], F32)
            bias_v = pp.tile([64, 1], F32)
            ident = pp.tile([128, 128], F32)

            make_identity(nc, ident[:, :])

            xt_v = xt[:, :].rearrange("p (j t) -> p j t", t=t_full)
            xts_v = xt_d.rearrange("(j p) t -> p j t", p=128)

            def load_slice(tq):
                nc.sync.dma_start(
                    out=xt_v[:, :, 512 * tq:512 * (tq + 1)],
                    in_=xts_v[:, :, 512 * tq:512 * (tq + 1)])

            # small operands on the scalar HWDGE queue (single contiguous DMAs)
            nc.scalar.dma_start(out=aux_sb[:, :], in_=aux_d)
            nc.scalar.dma_start(out=bias_kq[:, 0:1], in_=bkq_d[:, None])
            nc.scalar.dma_start(out=bias_v[:, 0:1], in_=bv_d[:, None])

            # V_aug padding: zero cols 64:128 of each slot, ones at col 64
            nc.gpsimd.memset(v_sb[:, :], 0.0)
            nc.vector.tensor_scalar(
                v_sb[:, :].rearrange("p (i c) -> p i c", c=128)[:, :, 64],
                ident[:, 0:n_own], 0.0, 1.0,
                op0=mybir.AluOpType.mult, op1=mybir.AluOpType.add)

            def project_kq(tq):
                ps = psb.tile([128, 512], F32, tag="psB")
                rhs = xt_v[:, :, 512 * tq:512 * (tq + 1)]
                for j in range(n_cchunk):
                    nc.tensor.ldweights(wkq_sb[:, 128 * j:128 * (j + 1)])
                    nc.tensor.matmul(
                        ps[:, :], wkq_sb[:, 128 * j:128 * (j + 1)],
                        rhs[:, j], start=(j == 0), stop=(j == n_cchunk - 1))
                nc.vector.tensor_scalar_add(
                    kqT[:, 512 * tq:512 * (tq + 1)], ps[:, :], bias_kq[:, 0:1])
                # S^T duplicate into partitions 64:128 for row-packed scores
                # (gpsimd SWDGE: the sync queue is busy streaming x)
                nc.gpsimd.dma_start(
                    sd_hi[64:128, 512 * tq:512 * (tq + 1)],
                    kqT[0:64, 512 * tq:512 * (tq + 1)])
                if tq < n_vq:
                    # G^T duplicate into partitions 0:64 (own region only)
                    nc.gpsimd.dma_start(
                        gt_lo[:, 512 * tq:512 * (tq + 1)],
                        kqT[64:128, 512 * tq:512 * (tq + 1)])

            def project_v(tq):
                ps = psb.tile([64, 512], F32, tag="psB")
                rhs = xt_v[:, :, 512 * tq:512 * (tq + 1)]
                for j in range(n_cchunk):
                    nc.tensor.matmul(
                        ps[:, :], wv_sb[:, H * j:H * (j + 1)],
                        rhs[:, j], start=(j == 0), stop=(j == n_cchunk - 1))
                nc.vector.tensor_scalar_add(
                    vT[:, 512 * tq:512 * (tq + 1)], ps[:, :], bias_v[:, 0:1])

            def v_transpose(i):
                ps = psb.tile([128, 64], F32, tag="psB")
                nc.tensor.transpose(
                    ps[:, :], vT[:, 128 * i:128 * (i + 1)], ident[0:64, 0:64])
                nc.vector.tensor_copy(v_sb[:, 128 * i:128 * i + 64], ps[:, :])

            # ---- loads + projections, own/partner slices interleaved so
            # query block tb only needs slices {tb//2, n_vq + tb//2} ----
            order = []
            for k in range(n_vq):
                order += [k, n_vq + k]
            for tq in order:
                load_slice(tq)

            kq_lo = kqT[0:64, :].rearrange("p (h t) -> p h t", h=2)
            sd_v = sd_hi[64:128, :].rearrange("p (h t) -> p h t", h=2)

            def attention_block(tb):
                poa = psoa.tile([128, 512], F32, tag="psOA")
                pob = psob.tile([128, 512], F32, tag="psOB")
                for ip in range(tb + 1):
                    i0, i1 = 2 * ip, 2 * ip + 1
                    ps = pss.tile([128, 1024], F32, tag="psS")
                    pt = ptp.tile([128, 1024], BF16, tag="pt")
                    nc.tensor.matmul(
                        ps[:, 0:512],
                        gt_lo[:, 128 * i0:128 * (i0 + 1)],
                        kq_lo[:, :, 256 * tb:256 * (tb + 1)],
                        start=True, stop=True, tile_position=(0, 0))
                    nc.tensor.matmul(
                        ps[:, 512:1024],
                        kqT[64:128, 128 * i1:128 * (i1 + 1)],
                        sd_v[:, :, 256 * tb:256 * (tb + 1)],
                        start=True, stop=True, tile_position=(64, 0))
                    nc.scalar.activation(
                        pt[:, :], ps[:, :],
                        mybir.ActivationFunctionType.Exp, scale=SCALE)
                    if ip == tb:
                        nc.vector.tensor_mul(
                            pt[:, 0:512], pt[:, 0:512], mask0)
                        nc.vector.tensor_mul(
                            pt[:, 512:1024], pt[:, 512:1024], mask1)
                    nc.tensor.ldweights(v_sb[:, 128 * i0:128 * (i0 + 1)])
                    nc.tensor.matmul(
                        poa[:, :], v_sb[:, 128 * i0:128 * (i0 + 1)],
                        pt[:, 0:512], start=(ip == 0), stop=(ip == tb))
                    nc.tensor.ldweights(v_sb[:, 128 * i1:128 * (i1 + 1)])
                    nc.tensor.matmul(
                        pob[:, :], v_sb[:, 128 * i1:128 * (i1 + 1)],
                        pt[:, 512:1024], start=(ip == 0), stop=(ip == tb))
                ob = outp.tile([65, 512], F32, tag="ob")
                nc.vector.tensor_copy(ob[:, :], poa[0:65, :])
                nc.vector.tensor_add(ob[:, :], pob[0:65, :], ob[:, :])
                nc.gpsimd.dma_start(
                    out=out_d[:, 512 * tb:512 * (tb + 1)], in_=ob[:, :])

            # staged pipeline: projections for slice-pair k, then the two
            # query blocks whose dependencies those slices complete
            for k in range(n_vq):
                project_kq(k)
                project_kq(n_vq + k)
                project_v(k)
                for i in range(4 * k, min(4 * (k + 1), n_own)):
                    v_transpose(i)
                if k == n_vq - 1:
                    attention_block(2 * k + 1)
                    attention_block(2 * k)
                else:
                    attention_block(2 * k)
                    attention_block(2 * k + 1)

    nc.compile()
    return nc


def make_core_inputs(x, Wk, bk, Wq, bq, Wv, bv, t_full=T):
    """Shard FULL inputs into the 8 per-core input dicts (layout prep only)."""
    n_tiles = t_full // 128
    n_cchunk = C // 128
    Wk = np.asarray(Wk, np.float32)
    Wq = np.asarray(Wq, np.float32)
    Wv = np.asarray(Wv, np.float32)
    wkq = np.empty((128, n_cchunk * 128), np.float32)
    wvf = np.empty((128, n_cchunk * H), np.float32)
    for j in range(n_cchunk):
        wkq[:, 128 * j:128 * j + 64] = Wk[128 * j:128 * (j + 1), :]
        wkq[:, 128 * j + 64:128 * (j + 1)] = Wq[128 * j:128 * (j + 1), :]
        wvf[:, H * j:H * (j + 1)] = Wv[128 * j:128 * (j + 1), :]
    bkq = np.concatenate([np.asarray(bk, np.float32),
                          np.asarray(bq, np.float32)])
    ins = []
    for core in range(N_CORES):
        b, p = core // 2, core % 2
        own = np.concatenate(
            [x[b, 128 * j:128 * (j + 1), :] for j in range(p, n_tiles, 2)]
            + [x[b, 128 * j:128 * (j + 1), :]
               for j in range(1 - p, n_tiles, 2)],
            axis=0)
        # mask[m][r, c]: own s-tile (local parity m, abs tile 4tb+2m+p) vs
        # query sub-tile c//128 (abs tile 4tb + A[c//128]); valid iff s <= t
        A = [p, 2 + p, 1 - p, 3 - p]
        masks = np.zeros((2, 128, 512), np.float32)
        rr = np.arange(128)[:, None]
        for m in (0, 1):
            for sub in range(4):
                cz = np.arange(128)[None, :]
                s_abs = 128 * (2 * m + p) + rr
                t_abs = 128 * A[sub] + cz
                masks[m, :, 128 * sub:128 * (sub + 1)] = (s_abs <= t_abs)
        aux = np.concatenate([wkq, wvf, masks[0], masks[1]], axis=1)
        ins.append({
            "xt_own": np.ascontiguousarray(own.T).astype(ml_dtypes.bfloat16),
            "aux": aux.astype(ml_dtypes.bfloat16),
            "bkq": bkq, "bv": np.asarray(bv, np.float32),
        })
    return ins


def _col_perm(p, t_full):
    """stored column -> absolute t index for a core with parity p."""
    A = [p, 2 + p, 1 - p, 3 - p]
    perm = np.empty(t_full, np.int64)
    for tb in range(t_full // 512):
        for sub in range(4):
            a = 128 * (4 * tb + A[sub])
            s = 512 * tb + 128 * sub
            perm[s:s + 128] = np.arange(a, a + 128)
    return perm


def combine_outputs(parts, t_full=T):
    """parts: list of 8 arrays [H+1, t_full] -> full output [B, t_full, H]."""
    out = np.empty((B, t_full, H), np.float32)
    for b in range(B):
        acc = np.zeros((H + 1, t_full), np.float32)
        for p in (0, 1):
            perm = _col_perm(p, t_full)
            acc[:, perm] += parts[2 * b + p]
        out[b] = (acc[:H, :] / acc[H:H + 1, :]).T
    return out


_NC_CACHE = {}


def kernel(x, Wk, bk, Wq, bq, Wv, bv):
    x = np.asarray(x, np.float32)
    t_full = x.shape[1]
    if t_full not in _NC_CACHE:
        _NC_CACHE[t_full] = build_kernel(t_full)
    nc = _NC_CACHE[t_full]
    ins = make_core_inputs(x, Wk, bk, Wq, bq, Wv, bv, t_full)
    res = run_bass_kernel_spmd(nc, ins, list(range(N_CORES)))
    parts = [res.results[i]["out_part"] for i in range(N_CORES)]
    return combine_outputs(parts, t_full)


if __name__ == "__main__":
    rng = np.random.default_rng(0)
    x = rng.standard_normal((B, T, C), dtype=np.float32)
    Wk = rng.standard_normal((C, H), dtype=np.float32) * SCALE
    Wq = rng.standard_normal((C, H), dtype=np.float32) * SCALE
    Wv = rng.standard_normal((C, H), dtype=np.float32) * SCALE
    bk = rng.standard_normal(H).astype(np.float32) * 0.02
    bq = rng.standard_normal(H).astype(np.float32) * 0.02
    bv = rng.standard_normal(H).astype(np.float32) * 0.02
    out = kernel(x=x, Wk=Wk, bk=bk, Wq=Wq, bq=bq, Wv=Wv, bv=bv)
    print(out.shape, out.dtype)


# revision 25
# speedup vs baseline: 1.0475x; 1.0475x over previous
"""Causal single-head attention (B=4, T=4096, C=1024, H=64) on 8 trn2 NeuronCores.

Sharding: core = (batch b = core//2, parity p = core%2). Each core owns the
interleaved context tiles {p, p+2, ...} of its batch (balanced under the causal
mask) and computes partial flash-attention (numerator + denominator) for ALL
queries of the batch; the host sums the two partials per batch and divides.

Host-side prep is layout only: the x shard is passed pre-transposed [C, T]
with columns ordered [own tiles | partner tiles] (pure permutation), weights
pre-fused into their SBUF layout, causal masks as data. All numerics run on
device. No collectives: each core projects the k-side for the full sequence
locally (trades HBM reads for the multi-10us cold-start of on-chip
collectives).

Device pipeline per core:
  load x^T slices (interleaved own/partner order) -> project [S^|G^] with a
  fused [wk|wq] fp32r stationary as slices land -> per 512-query block:
  row-packed bf16 score matmuls -> exp on ACT (scale folded in) -> data-driven
  causal masks on DVE -> bf16 PV matmuls (V padded to 128 cols for fast
  weight load) accumulating [V|1]^T @ P^T in dual PSUM chains -> write
  O^T_aug [H+1, T].

Query columns are processed in [own|partner] order per 512-block; the host
maps them back to absolute order per core in combine_outputs.
"""

import sys

for _p in ("/root/.axon_site/_ro/trn_rl_repo", "/root/.axon_site/_ro/pypackages"):
    if _p not in sys.path:
        sys.path.append(_p)

import ml_dtypes
import numpy as np

import concourse.bass as bass
import concourse.mybir as mybir
import concourse.tile as tile
from concourse import bacc
from concourse.bass_utils import run_bass_kernel_spmd
from concourse.masks import make_identity

B, T, C, H = 4, 4096, 1024, 64
N_CORES = 8
SCALE = C ** -0.5
F32 = mybir.dt.float32
F32R = mybir.dt.float32r
BF16 = mybir.dt.bfloat16


def build_kernel(t_full=T):
    """Build the SPMD Bass/Tile program for sequence length t_full."""
    t_own = t_full // 2           # context rows owned by this core
    n_own = t_own // 128          # own 128-row s-tiles
    n_blk = t_full // 512         # 512-wide query blocks
    n_cchunk = C // 128           # contraction chunks of 128
    n_tq = t_full // 512          # projection/load slices (stored order)
    n_vq = t_own // 512           # V projection slices (own region)

    nc = bacc.Bacc("TRN2", target_bir_lowering=False, debug=False,
                   num_devices=N_CORES)

    xt_d = nc.dram_tensor("xt_own", [C, t_full], BF16,
                      kind="ExternalInput").ap()
    aux_w = n_cchunk * 128 + n_cchunk * H + 1024
    aux_d = nc.dram_tensor("aux", [128, aux_w], BF16,
                           kind="ExternalInput").ap()
    bkq_d = nc.dram_tensor("bkq", [128], F32, kind="ExternalInput").ap()
    bv_d = nc.dram_tensor("bv", [64], F32, kind="ExternalInput").ap()
    out_d = nc.dram_tensor("out_part", [H + 1, t_full], F32,
                           kind="ExternalOutput").ap()

    with tile.TileContext(nc) as tc:
        with (
            tc.tile_pool(name="persist", bufs=1) as pp,
            tc.tile_pool(name="psB", bufs=2, space="PSUM") as psb,
            tc.tile_pool(name="psS", bufs=2, space="PSUM") as pss,
            tc.tile_pool(name="psOA", bufs=1, space="PSUM") as psoa,
            tc.tile_pool(name="psOB", bufs=1, space="PSUM") as psob,
            tc.tile_pool(name="ptp", bufs=4) as ptp,
            tc.tile_pool(name="outp", bufs=2) as outp,
        ):
            # ---- persistent SBUF tensors ----
            xt = pp.tile([128, n_cchunk * t_full], BF16)  # x^T, chunk j at cols t_full*j
            kqT = pp.tile([128, t_full], BF16)            # 0:64 = S^T, 64:128 = G^T
            sd_hi = pp.tile([128, t_full], BF16)          # S^T dup at partitions 64:128
            gt_lo = pp.tile([64, t_own], BF16)            # G^T (own) at partitions 0:64
            vT = pp.tile([64, t_own], F32)                # V^T own
            v_sb = pp.tile([128, n_own * 128], BF16)      # V_aug tiles [128,128] (padded)
            aux_sb = pp.tile([128, aux_w], BF16)
            wkq_sb = aux_sb[:, 0:n_cchunk * 128]
            wv_sb = aux_sb[:, n_cchunk * 128:n_cchunk * 128 + n_cchunk * H]
            mask0 = aux_sb[:, aux_w - 1024:aux_w - 512]
            mask1 = aux_sb[:, aux_w - 512:aux_w]
            bias_kq = pp.tile([128, 1], F32)
            bias_v = pp.tile([64, 1], F32)
            ident = pp.tile([128, 128], F32)

            make_identity(nc, ident[:, :])

            xt_v = xt[:, :].rearrange("p (j t) -> p j t", t=t_full)
            xts_v = xt_d.rearrange("(j p) t -> p j t", p=128)

            def load_slice(tq):
                nc.sync.dma_start(
                    out=xt_v[:, :, 512 * tq:512 * (tq + 1)],
                    in_=xts_v[:, :, 512 * tq:512 * (tq + 1)])

            # small operands on the scalar HWDGE queue (single contiguous DMAs)
            nc.scalar.dma_start(out=aux_sb[:, :], in_=aux_d)
            nc.scalar.dma_start(out=bias_kq[:, 0:1], in_=bkq_d[:, None])
            nc.scalar.dma_start(out=bias_v[:, 0:1], in_=bv_d[:, None])

            # V_aug padding: zero cols 64:128 of each slot, ones at col 64
            nc.gpsimd.memset(v_sb[:, :], 0.0)
            nc.vector.tensor_scalar(
                v_sb[:, :].rearrange("p (i c) -> p i c", c=128)[:, :, 64],
                ident[:, 0:n_own], 0.0, 1.0,
                op0=mybir.AluOpType.mult, op1=mybir.AluOpType.add)

            def project_kq(tq):
                ps = psb.tile([128, 512], F32, tag="psB")
                rhs = xt_v[:, :, 512 * tq:512 * (tq + 1)]
                for j in range(n_cchunk):
                    nc.tensor.matmul(
                        ps[:, :], wkq_sb[:, 128 * j:128 * (j + 1)],
                        rhs[:, j], start=(j == 0), stop=(j == n_cchunk - 1))
                nc.vector.tensor_scalar_add(
                    kqT[:, 512 * tq:512 * (tq + 1)], ps[:, :], bias_kq[:, 0:1])
                # S^T duplicate into partitions 64:128 for row-packed scores
                # (gpsimd SWDGE: the sync queue is busy streaming x)
                nc.gpsimd.dma_start(
                    sd_hi[64:128, 512 * tq:512 * (tq + 1)],
                    kqT[0:64, 512 * tq:512 * (tq + 1)])
                if tq < n_vq:
                    # G^T duplicate into partitions 0:64 (own region only)
                    nc.gpsimd.dma_start(
                        gt_lo[:, 512 * tq:512 * (tq + 1)],
                        kqT[64:128, 512 * tq:512 * (tq + 1)])

            def project_v(tq):
                ps = psb.tile([64, 512], F32, tag="psB")
                rhs = xt_v[:, :, 512 * tq:512 * (tq + 1)]
                for j in range(n_cchunk):
                    nc.tensor.matmul(
                        ps[:, :], wv_sb[:, H * j:H * (j + 1)],
                        rhs[:, j], start=(j == 0), stop=(j == n_cchunk - 1))
                nc.vector.tensor_scalar_add(
                    vT[:, 512 * tq:512 * (tq + 1)], ps[:, :], bias_v[:, 0:1])

            def v_transpose(i):
                ps = psb.tile([128, 64], F32, tag="psB")
                nc.tensor.transpose(
                    ps[:, :], vT[:, 128 * i:128 * (i + 1)], ident[0:64, 0:64])
                nc.vector.tensor_copy(v_sb[:, 128 * i:128 * i + 64], ps[:, :])

            # ---- loads + projections, own/partner slices interleaved so
            # query block tb only needs slices {tb//2, n_vq + tb//2} ----
            order = []
            for k in range(n_vq):
                order += [k, n_vq + k]
            for tq in order:
                load_slice(tq)

            kq_lo = kqT[0:64, :].rearrange("p (h t) -> p h t", h=2)
            sd_v = sd_hi[64:128, :].rearrange("p (h t) -> p h t", h=2)

            def attention_block(tb):
                poa = psoa.tile([128, 512], F32, tag="psOA")
                pob = psob.tile([128, 512], F32, tag="psOB")
                for ip in range(tb + 1):
                    i0, i1 = 2 * ip, 2 * ip + 1
                    ps = pss.tile([128, 1024], F32, tag="psS")
                    pt = ptp.tile([128, 1024], BF16, tag="pt")
                    nc.tensor.matmul(
                        ps[:, 0:512],
                        gt_lo[:, 128 * i0:128 * (i0 + 1)],
                        kq_lo[:, :, 256 * tb:256 * (tb + 1)],
                        start=True, stop=True, tile_position=(0, 0))
                    nc.tensor.matmul(
                        ps[:, 512:1024],
                        kqT[64:128, 128 * i1:128 * (i1 + 1)],
                        sd_v[:, :, 256 * tb:256 * (tb + 1)],
                        start=True, stop=True, tile_position=(64, 0))
                    nc.scalar.activation(
                        pt[:, :], ps[:, :],
                        mybir.ActivationFunctionType.Exp, scale=SCALE)
                    if ip == tb:
                        nc.vector.tensor_mul(
                            pt[:, 0:512], pt[:, 0:512], mask0)
                        nc.vector.tensor_mul(
                            pt[:, 512:1024], pt[:, 512:1024], mask1)
                    nc.tensor.matmul(
                        poa[:, :], v_sb[:, 128 * i0:128 * (i0 + 1)],
                        pt[:, 0:512], start=(ip == 0), stop=(ip == tb))
                    nc.tensor.matmul(
                        pob[:, :], v_sb[:, 128 * i1:128 * (i1 + 1)],
                        pt[:, 512:1024], start=(ip == 0), stop=(ip == tb))
                ob = outp.tile([65, 512], F32, tag="ob")
                nc.vector.tensor_copy(ob[:, :], poa[0:65, :])
                nc.vector.tensor_add(ob[:, :], pob[0:65, :], ob[:, :])
                nc.gpsimd.dma_start(
                    out=out_d[:, 512 * tb:512 * (tb + 1)], in_=ob[:, :])

            # staged pipeline: projections for slice-pair k, then the two
            # query blocks whose dependencies those slices complete
            for k in range(n_vq):
                project_kq(k)
                project_kq(n_vq + k)
                project_v(k)
                for i in range(4 * k, min(4 * (k + 1), n_own)):
                    v_transpose(i)
                if k == n_vq - 1:
                    attention_block(2 * k + 1)
                    attention_block(2 * k)
                else:
                    attention_block(2 * k)
                    attention_block(2 * k + 1)

    nc.compile()
    return nc


def make_core_inputs(x, Wk, bk, Wq, bq, Wv, bv, t_full=T):
    """Shard FULL inputs into the 8 per-core input dicts (layout prep only)."""
    n_tiles = t_full // 128
    n_cchunk = C // 128
    Wk = np.asarray(Wk, np.float32)
    Wq = np.asarray(Wq, np.float32)
    Wv = np.asarray(Wv, np.float32)
    wkq = np.empty((128, n_cchunk * 128), np.float32)
    wvf = np.empty((128, n_cchunk * H), np.float32)
    for j in range(n_cchunk):
        wkq[:, 128 * j:128 * j + 64] = Wk[128 * j:128 * (j + 1), :]
        wkq[:, 128 * j + 64:128 * (j + 1)] = Wq[128 * j:128 * (j + 1), :]
        wvf[:, H * j:H * (j + 1)] = Wv[128 * j:128 * (j + 1), :]
    bkq = np.concatenate([np.asarray(bk, np.float32),
                          np.asarray(bq, np.float32)])
    ins = []
    for core in range(N_CORES):
        b, p = core // 2, core % 2
        own = np.concatenate(
            [x[b, 128 * j:128 * (j + 1), :] for j in range(p, n_tiles, 2)]
            + [x[b, 128 * j:128 * (j + 1), :]
               for j in range(1 - p, n_tiles, 2)],
            axis=0)
        # mask[m][r, c]: own s-tile (local parity m, abs tile 4tb+2m+p) vs
        # query sub-tile c//128 (abs tile 4tb + A[c//128]); valid iff s <= t
        A = [p, 2 + p, 1 - p, 3 - p]
        masks = np.zeros((2, 128, 512), np.float32)
        rr = np.arange(128)[:, None]
        for m in (0, 1):
            for sub in range(4):
                cz = np.arange(128)[None, :]
                s_abs = 128 * (2 * m + p) + rr
                t_abs = 128 * A[sub] + cz
                masks[m, :, 128 * sub:128 * (sub + 1)] = (s_abs <= t_abs)
        aux = np.concatenate([wkq, wvf, masks[0], masks[1]], axis=1)
        ins.append({
            "xt_own": np.ascontiguousarray(own.T).astype(ml_dtypes.bfloat16),
            "aux": aux.astype(ml_dtypes.bfloat16),
            "bkq": bkq, "bv": np.asarray(bv, np.float32),
        })
    return ins


def _col_perm(p, t_full):
    """stored column -> absolute t index for a core with parity p."""
    A = [p, 2 + p, 1 - p, 3 - p]
    perm = np.empty(t_full, np.int64)
    for tb in range(t_full // 512):
        for sub in range(4):
            a = 128 * (4 * tb + A[sub])
            s = 512 * tb + 128 * sub
            perm[s:s + 128] = np.arange(a, a + 128)
    return perm


def combine_outputs(parts, t_full=T):
    """parts: list of 8 arrays [H+1, t_full] -> full output [B, t_full, H]."""
    out = np.empty((B, t_full, H), np.float32)
    for b in range(B):
        acc = np.zeros((H + 1, t_full), np.float32)
        for p in (0, 1):
            perm = _col_perm(p, t_full)
            acc[:, perm] += parts[2 * b + p]
        out[b] = (acc[:H, :] / acc[H:H + 1, :]).T
    return out


_NC_CACHE = {}


def kernel(x, Wk, bk, Wq, bq, Wv, bv):
    x = np.asarray(x, np.float32)
    t_full = x.shape[1]
    if t_full not in _NC_CACHE:
        _NC_CACHE[t_full] = build_kernel(t_full)
    nc = _NC_CACHE[t_full]
    ins = make_core_inputs(x, Wk, bk, Wq, bq, Wv, bv, t_full)
    res = run_bass_kernel_spmd(nc, ins, list(range(N_CORES)))
    parts = [res.results[i]["out_part"] for i in range(N_CORES)]
    return combine_outputs(parts, t_full)


if __name__ == "__main__":
    rng = np.random.default_rng(0)
    x = rng.standard_normal((B, T, C), dtype=np.float32)
    Wk = rng.standard_normal((C, H), dtype=np.float32) * SCALE
    Wq = rng.standard_normal((C, H), dtype=np.float32) * SCALE
    Wv = rng.standard_normal((C, H), dtype=np.float32) * SCALE
    bk = rng.standard_normal(H).astype(np.float32) * 0.02
    bq = rng.standard_normal(H).astype(np.float32) * 0.02
    bv = rng.standard_normal(H).astype(np.float32) * 0.02
    out = kernel(x=x, Wk=Wk, bk=bk, Wq=Wq, bq=bq, Wv=Wv, bv=bv)
    print(out.shape, out.dtype)
